# revision 1
# baseline (speedup 1.0000x reference)
"""CDFNormalizer (histogram binning) Trainium2 Bass kernel.

z[n,d] = LUT[searchsorted(quantiles[:,d], x[n,d], side='left')]
with LUT[j] = sqrt(2)*erfinv(2*clip(j/1023, eps, 1-eps)-1).

Device model (per dim d, z-space, no per-element table lookup — the
quantile staircase is approximated by a host-fitted degree-8 polynomial
plus greedy weighted step-knots, with the large tail steps handled
exactly by min/max cascades):

  t  = (x - mu_d) * inv_d
  h  = poly_d(t) + sum_k w_kd * H(x > s_kd)
  z  = clip(h, LUT[KL], LUT[1024-KR])
  z  = min(z, (M if x > q_jd else 0) + LUT[j])        j = 0..KL-1
  z  = max(z, (-M if x <= q_jd else 0) + LUT[j+1])    j = 1024-KR..1023

Data-parallel across 8 NeuronCores along the row axis. Layout on core:
contiguous DMA loads, TensorE 128x128 transposes to a dim-major layout
(partition = (row_chunk, dim)), fused DVE tensor_scalar /
scalar_tensor_tensor passes with per-partition constants, TensorE
transpose back.
"""

import math

import numpy as np

N = 2_097_152
D = 32
BINS = 1024
EPS = 1e-06
SQRT2 = 1.41421356
NCORES = 8
RPC = N // NCORES

TILE_ROWS = 8192
G = 64
TFREE = 2048
NTILES = RPC // TILE_ROWS

DEG = 8
KL = 6
KR = 6
NKNOT = 24
TAIL_ENGINE = "vector"
BIGM = 1.0e30

COL_INV = 0
COL_SHIFT = 1
COL_A = 2
COL_A1 = 3
COL_C0 = 3 + DEG - 1
COL_QL = COL_C0 + 1
COL_QR = COL_QL + KL
COL_QK = COL_QR + KR
COL_WK = COL_QK + NKNOT
NCONST = COL_WK + NKNOT


def _erfinv(y: float) -> float:
    if y <= -1.0:
        return -math.inf
    if y >= 1.0:
        return math.inf
    w = -math.log((1.0 - y) * (1.0 + y))
    if w < 5.0:
        w2 = w - 2.5
        p = 2.81022636e-08
        for c in (3.43273939e-07, -3.5233877e-06, -4.39150654e-06, 2.1858087e-04,
                  -1.25372503e-03, -4.17768164e-03, 2.46640727e-01, 1.50140941e00):
            p = p * w2 + c
        x = p * y
    else:
        w2 = math.sqrt(w) - 3.0
        p = -2.00214257e-04
        for c in (1.00950558e-04, 1.34934322e-03, -3.67342844e-03, 5.73950773e-03,
                  -7.62246130e-03, 9.43887047e-03, 1.00167406e00, 2.83297682e00):
            p = p * w2 + c
        x = p * y
    c2 = 2.0 / math.sqrt(math.pi)
    for _ in range(3):
        err = math.erf(x) - y
        x -= err / (c2 * math.exp(-x * x))
    return x


def _build_lut() -> np.ndarray:
    j = np.arange(BINS + 1, dtype=np.float64)
    u = np.clip(j / (BINS - 1), EPS, 1.0 - EPS)
    lut = np.array([_erfinv(2.0 * ui - 1.0) for ui in u], dtype=np.float64)
    return lut * SQRT2


def _bf16_eff_threshold(q: float) -> float:
    """x-threshold where (bf16(x) > q) flips, as fp64. Comparing bf16(x) > q
    equals comparing x > thr_eff with thr_eff returned here."""
    import ml_dtypes
    bf = ml_dtypes.bfloat16
    lo, hi = q - abs(q) * 0.01 - 1e-3, q + abs(q) * 0.01 + 1e-3
    f = lambda x: float(np.float32(x).astype(bf).astype(np.float64)) > q
    assert not f(lo) and f(hi)
    for _ in range(80):
        mid = 0.5 * (lo + hi)
        if f(mid):
            hi = mid
        else:
            lo = mid
    return hi


def _fit_dim(qd: np.ndarray, lutd: np.ndarray) -> dict:
    lo_x, hi_x = qd[KL - 1], qd[BINS - KR]
    mu = 0.5 * (lo_x + hi_x)
    inv = 2.0 / (hi_x - lo_x)
    bs = np.arange(KL, BINS - KR + 1)
    xm = 0.5 * (qd[bs - 1] + qd[bs])
    ym = lutd[bs]
    tm = (xm - mu) * inv
    nb = len(bs)
    V = np.vander(tm, DEG + 1, increasing=True)
    knot_bins: list[int] = []
    cols = [V]
    beta = None
    for it in range(NKNOT + 1):
        X = np.concatenate(cols, axis=1)
        beta, *_ = np.linalg.lstsq(X, ym, rcond=None)
        r = ym - X @ beta
        if it == NKNOT:
            break
        csum = np.cumsum(r[::-1])[::-1]
        cnt = np.arange(nb, 0, -1)
        gain = np.zeros(nb)
        gain[1:] = csum[1:] ** 2 / cnt[1:]
        for jb in knot_bins:
            i = jb - KL + 1
            gain[max(0, i - 1):i + 2] = 0
        i_star = int(np.argmax(gain))
        j_star = int(bs[i_star] - 1)
        knot_bins.append(j_star)
        cols.append((bs[:, None] > j_star).astype(np.float64))
    import ml_dtypes
    kw = beta[DEG + 1:]
    # device compares bf16(x) > q[j]; effective threshold in x-space + bf16 weights
    thr = [_bf16_eff_threshold(float(qd[j])) for j in knot_bins]
    kw_b = np.asarray(kw, np.float32).astype(ml_dtypes.bfloat16).astype(np.float64)
    # refit poly on residual with effective knot bases (on bin midpoints xm)
    resid = ym.copy()
    for th, w in zip(thr, kw_b):
        resid -= w * (xm > th)
    beta2, *_ = np.linalg.lstsq(V, resid, rcond=None)
    cs = beta2
    return {
        "mu": mu, "inv": inv, "A": cs[DEG],
        "a": [cs[DEG - i] for i in range(1, DEG)], "c0": cs[0],
        "qK": [float(qd[j]) for j in knot_bins], "wK": list(kw_b),
        "qL": [qd[j] for j in range(KL)],
        "qR": [qd[BINS - KR + j] for j in range(KR)],
    }


def _build_consts(quantiles: np.ndarray):
    lutd = _build_lut()
    fits = [_fit_dim(quantiles[:, d].astype(np.float64), lutd) for d in range(D)]
    cols = []

    def col(vals):
        cols.append(np.asarray(vals, dtype=np.float64))

    col([f["inv"] for f in fits])
    col([-f["mu"] * f["inv"] for f in fits])
    col([f["A"] for f in fits])
    for i in range(DEG - 1):
        col([f["a"][i] for f in fits])
    col([f["c0"] for f in fits])
    for j in range(KL):
        col([f["qL"][j] for f in fits])
    for j in range(KR):
        col([f["qR"][j] for f in fits])
    for k in range(NKNOT):
        col([f["qK"][k] for f in fits])
    for k in range(NKNOT):
        col([f["wK"][k] for f in fits])
    consts32 = np.stack(cols, axis=1)
    consts = np.tile(consts32, (4, 1)).astype(np.float32)
    imms = {
        "clampL": float(lutd[KL]),
        "clampH": float(lutd[BINS - KR]),
        "lutL": [float(lutd[j]) for j in range(KL)],
        "lutR": [float(lutd[BINS - KR + j + 1]) for j in range(KR)],
    }
    return consts, imms


def build_kernel(imms: dict, rpc: int = RPC, ntiles: int | None = None,
                 finalize: bool = True, repeat: int = 1):
    import concourse.bass as bass
    import concourse.mybir as mybir
    from concourse import bacc, tile

    if ntiles is None:
        ntiles = rpc // TILE_ROWS
    dt = mybir.dt.float32
    op = mybir.AluOpType

    nc = bacc.Bacc(None)
    x_ext = nc.declare_dram_parameter("x", [rpc, D], dt, isOutput=False)
    consts_ext = nc.declare_dram_parameter("consts", [128, NCONST], dt,
                                           isOutput=False)
    ident_ext = nc.declare_dram_parameter("ident", [128, 128], dt,
                                          isOutput=False)
    z_ext = nc.declare_dram_parameter("z", [rpc, D], dt, isOutput=True)

    x_view = x_ext.rearrange("(p g) d -> p (g d)", p=128)
    z_view = z_ext.rearrange("(p g) d -> p (g d)", p=128)

    with tile.TileContext(nc) as tc:
        with (
            tc.tile_pool(name="const", bufs=1) as cpool,
            tc.tile_pool(name="work", bufs=2) as wpool,
            tc.tile_pool(name="zw", bufs=2) as zpool,
            tc.tile_pool(name="pin", bufs=1, space="PSUM") as pin,
            tc.tile_pool(name="pout", bufs=1, space="PSUM") as pout,
        ):
            ct = cpool.tile([128, NCONST], dt, tag="consts")
            ident = cpool.tile([128, 128], dt, tag="ident")
            nc.sync.dma_start(ct[:], consts_ext[:])
            nc.sync.dma_start(ident[:], ident_ext[:])

            def sc(j):
                return ct[:, j:j + 1]

            gpt = G * D

            for _rep in range(repeat):
              for it in range(ntiles):
                  xn = wpool.tile([128, TFREE], dt, tag="xn")
                  nc.sync.dma_start(xn[:], x_view[:, it * gpt:(it + 1) * gpt])

                  xtp = pin.tile([128, TFREE], dt, tag="xt")
                  for k in range(TFREE // 128):
                      nc.tensor.transpose(xtp[:, k * 128:(k + 1) * 128],
                                          xn[:, k * 128:(k + 1) * 128], ident[:])

                  xs = wpool.tile([128, TFREE], dt, tag="xs")
                  nc.scalar.copy(xs[:], xtp[:])

                  xb = wpool.tile([128, TFREE], mybir.dt.bfloat16, tag="xb")
                  nc.scalar.copy(xb[:], xtp[:])

                  t = wpool.tile([128, TFREE], dt, tag="t")
                  nc.vector.tensor_scalar(t[:], xs[:], sc(COL_INV), sc(COL_SHIFT),
                                          op.mult, op.add)
                  h = wpool.tile([128, TFREE], dt, tag="h")
                  nc.vector.tensor_scalar(h[:], t[:], sc(COL_A), None, op.mult)
                  for i in range(DEG - 1):
                      nc.vector.scalar_tensor_tensor(h[:], h[:], sc(COL_A1 + i),
                                                     t[:], op.add, op.mult)
                  ub = wpool.tile([128, TFREE], mybir.dt.bfloat16, tag="ub")
                  ab = wpool.tile([128, TFREE], mybir.dt.bfloat16, tag="ab")
                  nc.vector.tensor_scalar(ab[:], xb[:], sc(COL_QK + 0),
                                          sc(COL_WK + 0), op.is_gt, op.mult)
                  for k in range(1, NKNOT):
                      nc.vector.tensor_scalar(ub[:], xb[:], sc(COL_QK + k),
                                              sc(COL_WK + k), op.is_gt, op.mult)
                      nc.vector.tensor_tensor(ab[:], ab[:], ub[:], op.add)
                  nc.vector.scalar_tensor_tensor(h[:], h[:], sc(COL_C0), ab[:],
                                                 op.add, op.add)
                  u = wpool.tile([128, TFREE], dt, tag="u")
                  z = zpool.tile([128, TFREE], dt, tag="z")
                  nc.vector.tensor_scalar(z[:], h[:], float(imms["clampL"]),
                                          float(imms["clampH"]), op.max, op.min)
                  teng = nc.gpsimd if TAIL_ENGINE == "gpsimd" else nc.vector
                  for j in range(KL):
                      teng.tensor_scalar(u[:], xs[:], sc(COL_QL + j), BIGM,
                                         op.is_gt, op.mult)
                      nc.vector.scalar_tensor_tensor(z[:], u[:],
                                                     float(imms["lutL"][j]),
                                                     z[:], op.add, op.min)
                  for j in range(KR):
                      teng.tensor_scalar(u[:], xs[:], sc(COL_QR + j), -BIGM,
                                         op.is_le, op.mult)
                      nc.vector.scalar_tensor_tensor(z[:], u[:],
                                                     float(imms["lutR"][j]),
                                                     z[:], op.add, op.max)

                  ztp = pout.tile([128, TFREE], dt, tag="zt")
                  for k in range(TFREE // 128):
                      nc.tensor.transpose(ztp[:, k * 128:(k + 1) * 128],
                                          z[:, k * 128:(k + 1) * 128], ident[:])
                  zs = zpool.tile([128, TFREE], dt, tag="zs")
                  nc.scalar.copy(zs[:], ztp[:])
                  nc.sync.dma_start(z_view[:, it * gpt:(it + 1) * gpt], zs[:])

    if finalize:
        nc.finalize()
    return nc


_CACHE: dict = {}


def kernel(x: np.ndarray, quantiles: np.ndarray) -> np.ndarray:
    from concourse.bass_utils import run_bass_kernel_spmd

    x = np.ascontiguousarray(np.asarray(x, dtype=np.float32))
    quantiles = np.ascontiguousarray(np.asarray(quantiles, dtype=np.float32))
    assert x.shape == (N, D) and quantiles.shape == (BINS, D)

    consts, imms = _build_consts(quantiles)
    key = "nc"
    if key not in _CACHE:
        _CACHE[key] = build_kernel(imms)
    nc = _CACHE[key]

    ident = np.eye(128, dtype=np.float32)
    core_ids = list(range(NCORES))
    in_maps = [
        {"x": x[c * RPC:(c + 1) * RPC], "consts": consts, "ident": ident}
        for c in core_ids
    ]
    res = run_bass_kernel_spmd(nc, in_maps, core_ids)
    out = np.concatenate([res.results[i]["z"] for i in range(NCORES)], axis=0)
    return out.astype(np.float32)

